# revision 1
# baseline (speedup 1.0000x reference)
"""Trainium2 Bass kernel for nn_EquivariantProductBasisWithSelfMagmomBlock.

Data-parallel over nodes: 8 NeuronCores x 8192 nodes each.
Per-core layout: nodes-on-partition for elementwise, PE transposes to
channel-on-partition for the output linear layers (float32r matmuls).
Node map inside a core: local node n = p*64 + t  (p = partition, t = tile).
"""

import sys

sys.path.insert(0, "/opt/trn_rl_repo")

from contextlib import ExitStack

import numpy as np

import concourse.bass as bass
import concourse.tile as tile
from concourse import bacc, mybir
from concourse.bass_utils import run_bass_kernel_spmd
from concourse.masks import make_identity

FP32 = mybir.dt.float32
F32R = mybir.dt.float32r
AF = mybir.ActivationFunctionType
OP = mybir.AluOpType

N = 65536
C = 128
E = 10
INV = 16
N_CORES = 8
N_CORE = N // N_CORES  # 8192
P = 128


def r(ap):
    """bitcast an AP to float32r for full-rate fp32 matmul."""
    return ap.bitcast(F32R)


def build_program(n_tiles):
    """Build the per-core SPMD program. n_tiles tiles of 128 nodes each."""
    nc = bacc.Bacc(
        "TRN2", target_bir_lowering=False, debug=False, num_devices=N_CORES
    )
    n_nodes = n_tiles * P

    def din(name, shape):
        return nc.dram_tensor(name, list(shape), FP32, kind="ExternalInput").ap()

    nf_d = din("node_feats", (n_nodes, 4 * C))
    sc_d = din("sc", (n_nodes, 4 * C))
    attrs_d = din("node_attrs", (n_nodes, E))
    inv_d = din("magmom_node_inv_feats", (n_nodes, INV))
    mag_d = din("magmom_node_attrs", (n_nodes, 4))
    wsc0_d = din("w_sc0", (E, 5 * C))
    wsc1_d = din("w_sc1", (E, 4 * C))
    w1_d = din("w_mlp1", (INV, 64))
    w2_d = din("w_mlp2", (64, 64))
    w3_d = din("w_mlp3", (64, 64))
    w4_d = din("w_mlp4", (64, 4 * C))
    wl0_d = din("W_l0", (2 * C, C))
    wl1_d = din("W_l1", (2 * C, C))
    wo0_d = din("Wo0", (C, C))
    wo1_d = din("Wo1", (C, C))
    out_d = nc.dram_tensor("out", [n_nodes, 4 * C], FP32, kind="ExternalOutput").ap()

    # node n = s*512 + p*4 + q  <->  (supertile s, partition p, quarter q)
    assert n_tiles % 4 == 0
    n_st = n_tiles // 4
    nf_r = nf_d.rearrange("(s p q) x -> p s (q x)", p=P, q=4)
    sc_r = sc_d.rearrange("(s p q) x -> p s (q x)", p=P, q=4)
    out_r = out_d.rearrange("(s p q) x -> p s (q x)", p=P, q=4)
    attrs_r = attrs_d.rearrange("(s p q) x -> p s q x", p=P, q=4)
    inv_r = inv_d.rearrange("(s p q) x -> p s q x", p=P, q=4)
    mag_r = mag_d.rearrange("(s p q) x -> p s q x", p=P, q=4)

    with tile.TileContext(nc) as tc, ExitStack() as ctx:
        singles = ctx.enter_context(tc.tile_pool(name="singles", bufs=1))
        nat = ctx.enter_context(tc.tile_pool(name="nat", bufs=3))
        ew = ctx.enter_context(tc.tile_pool(name="ew", bufs=2))
        prodT = ctx.enter_context(tc.tile_pool(name="prodT", bufs=2))
        small = ctx.enter_context(tc.tile_pool(name="small", bufs=2))
        # PSUM pools (8 banks total):
        # wz [128,1280] = 2.5 -> 3 banks; tpw 1; mlp 1; T 2 (2x 1-bank); o 1
        wz_ps_pool = ctx.enter_context(tc.tile_pool(name="wz_ps", bufs=1, space="PSUM"))
        tpw_ps_pool = ctx.enter_context(
            tc.tile_pool(name="tpw_ps", bufs=1, space="PSUM")
        )
        t_ps_pool = ctx.enter_context(tc.tile_pool(name="t_ps", bufs=2, space="PSUM"))
        o_ps_pool = ctx.enter_context(tc.tile_pool(name="o_ps", bufs=1, space="PSUM"))

        # ---------------- preloads ----------------
        ident = singles.tile([P, P], FP32)
        make_identity(nc, ident[:])

        attrs_all = singles.tile([P, n_st, 4, E], FP32)
        nc.sync.dma_start(out=attrs_all[:], in_=attrs_r)
        inv_all = singles.tile([P, n_st, 4, INV], FP32)
        nc.sync.dma_start(out=inv_all[:], in_=inv_r)
        mag_all = singles.tile([P, n_st, 4, 4], FP32)
        nc.sync.dma_start(out=mag_all[:], in_=mag_r)

        wscf = singles.tile([E, 1280], FP32)
        nc.vector.memset(wscf[:, 1152:1280], 0.0)
        nc.sync.dma_start(out=wscf[:, 0:640], in_=wsc0_d)
        nc.sync.dma_start(out=wscf[:, 640:1152], in_=wsc1_d)
        wsc = singles.tile([E, 1280], F32R)
        nc.vector.tensor_copy(wsc[:], wscf[:])

        w1f = singles.tile([INV, 64], FP32)
        nc.sync.dma_start(out=w1f[:], in_=w1_d)
        w2f = singles.tile([64, 64], FP32)
        nc.sync.dma_start(out=w2f[:], in_=w2_d)
        w3f = singles.tile([64, 64], FP32)
        nc.sync.dma_start(out=w3f[:], in_=w3_d)
        w4f = singles.tile([64, 4 * C], FP32)
        nc.sync.dma_start(out=w4f[:], in_=w4_d)
        w1 = singles.tile([INV, 64], F32R)
        nc.vector.tensor_copy(w1[:], w1f[:])
        w2 = singles.tile([64, 64], F32R)
        nc.vector.tensor_copy(w2[:], w2f[:])
        w3 = singles.tile([64, 64], F32R)
        nc.vector.tensor_copy(w3[:], w3f[:])
        w4 = singles.tile([64, 4 * C], F32R)
        nc.vector.tensor_copy(w4[:], w4f[:])

        Wf = singles.tile([P, 6, C], FP32)
        nc.sync.dma_start(out=Wf[:, 0, :], in_=wl0_d[0:128, :])
        nc.sync.dma_start(out=Wf[:, 1, :], in_=wl0_d[128:256, :])
        nc.sync.dma_start(out=Wf[:, 2, :], in_=wl1_d[0:128, :])
        nc.sync.dma_start(out=Wf[:, 3, :], in_=wl1_d[128:256, :])
        nc.sync.dma_start(out=Wf[:, 4, :], in_=wo0_d)
        nc.sync.dma_start(out=Wf[:, 5, :], in_=wo1_d)
        WA0, WB0, WA1, WB1, WO0, WO1 = (Wf[:, k, :] for k in range(6))

        for s_ in range(n_st):
            # ---------------- supertile loads (1 MB each) ----------------
            nf_st = nat.tile([P, 16 * C], FP32, tag="nf")
            nc.sync.dma_start(out=nf_st[:], in_=nf_r[:, s_, :])
            sc_st = nat.tile([P, 16 * C], FP32, tag="sc")
            nc.sync.dma_start(out=sc_st[:], in_=sc_r[:, s_, :])
            out_st = nat.tile([P, 16 * C], FP32, tag="out")
            for q in range(4):
                nf_t = nf_st[:, q * 4 * C : (q + 1) * 4 * C]
                sc_t = sc_st[:, q * 4 * C : (q + 1) * 4 * C]
                out_t = out_st[:, q * 4 * C : (q + 1) * 4 * C]
                nfv = nf_t.rearrange("p (c j) -> p c j", j=4)
                x0 = nfv[:, :, 0]
                x1 = [nfv[:, :, 1 + m] for m in range(3)]
                a0 = mag_all[:, s_, q, 0:1]
                a1 = [mag_all[:, s_, q, 1 + m : 2 + m] for m in range(3)]

                # ---------------- small transposes ----------------
                tp = t_ps_pool.tile([P, P], FP32, tag="tps")
                nc.tensor.transpose(tp[0:E, :], attrs_all[:, s_, q, :], ident[:])
                attrs_T = small.tile([E, P], F32R, tag="attrsT")
                nc.scalar.copy(attrs_T[:], tp[0:E, :])

                tp2 = t_ps_pool.tile([P, P], FP32, tag="tps")
                nc.tensor.transpose(tp2[0:INV, :], inv_all[:, s_, q, :], ident[:])
                inv_T = small.tile([INV, P], F32R, tag="invT")
                nc.scalar.copy(inv_T[:], tp2[0:INV, :])

                # ---------------- wz selection matmul ----------------
                wz = wz_ps_pool.tile([P, 1280], FP32, tag="wz")
                nc.tensor.matmul(wz[:, 0:512], attrs_T[:], wsc[:, 0:512])
                nc.tensor.matmul(wz[:, 512:1024], attrs_T[:], wsc[:, 512:1024])
                nc.tensor.matmul(wz[:, 1024:1280], attrs_T[:], wsc[:, 1024:1280])
                wz0 = [wz[:, p * C : (p + 1) * C] for p in range(5)]
                wz1 = [wz[:, 640 + p * C : 640 + (p + 1) * C] for p in range(4)]

                # ---------------- magmom MLP ----------------
                hps = t_ps_pool.tile([P, 3, 64], FP32, tag="tps")
                nc.tensor.matmul(hps[:, 0, :], inv_T[:], w1[:])
                sg1 = small.tile([P, 64], FP32, tag="sg1")
                nc.scalar.activation(sg1[:], hps[:, 0, :], AF.Sigmoid)
                h1 = small.tile([P, 64], FP32, tag="h1")
                nc.vector.tensor_mul(h1[:], hps[:, 0, :], sg1[:])
                tph = t_ps_pool.tile([P, P], FP32, tag="tps")
                nc.tensor.transpose(tph[0:64, :], h1[:], ident[:])
                h1T = small.tile([64, P], F32R, tag="h1T")
                nc.scalar.copy(h1T[:], tph[0:64, :])

                nc.tensor.matmul(hps[:, 1, :], h1T[:], w2[:])
                sg2 = small.tile([P, 64], FP32, tag="sg2")
                nc.scalar.activation(sg2[:], hps[:, 1, :], AF.Sigmoid)
                h2 = small.tile([P, 64], FP32, tag="h2")
                nc.vector.tensor_mul(h2[:], hps[:, 1, :], sg2[:])
                tph2 = t_ps_pool.tile([P, P], FP32, tag="tps")
                nc.tensor.transpose(tph2[0:64, :], h2[:], ident[:])
                h2T = small.tile([64, P], F32R, tag="h2T")
                nc.scalar.copy(h2T[:], tph2[0:64, :])

                nc.tensor.matmul(hps[:, 2, :], h2T[:], w3[:])
                sg3 = small.tile([P, 64], FP32, tag="sg3")
                nc.scalar.activation(sg3[:], hps[:, 2, :], AF.Sigmoid)
                h3 = small.tile([P, 64], FP32, tag="h3")
                nc.vector.tensor_mul(h3[:], hps[:, 2, :], sg3[:])
                tph3 = t_ps_pool.tile([P, P], FP32, tag="tps")
                nc.tensor.transpose(tph3[0:64, :], h3[:], ident[:])
                h3T = small.tile([64, P], F32R, tag="h3T")
                nc.scalar.copy(h3T[:], tph3[0:64, :])

                tpw = tpw_ps_pool.tile([P, 4 * C], FP32, tag="tpw")
                nc.tensor.matmul(tpw[:], h3T[:], w4[:])
                wa = tpw[:, 0:C]
                wb = tpw[:, C : 2 * C]
                wc = tpw[:, 2 * C : 3 * C]
                wd = tpw[:, 3 * C : 4 * C]

                # ---------------- elementwise: monomials ----------------
                sq = ew.tile([P, 4 * C], FP32, tag="sq")
                nc.scalar.activation(sq[:], nf_t, AF.Square)
                sqv = sq[:].rearrange("p (c j) -> p c j", j=4)
                x0sq = sqv[:, :, 0]

                n1 = ew.tile([P, C], FP32, tag="n1")
                nc.gpsimd.tensor_add(n1[:], sqv[:, :, 1], sqv[:, :, 2])
                nc.gpsimd.tensor_add(n1[:], n1[:], sqv[:, :, 3])

                x0cu = ew.tile([P, C], FP32, tag="x0cu")
                nc.gpsimd.tensor_mul(x0cu[:], x0sq, x0)
                x0n1 = ew.tile([P, C], FP32, tag="x0n1")
                nc.vector.tensor_mul(x0n1[:], x0, n1[:])

                # ---------------- y0 = sum_p wz0_p * m_p ----------------
                y0 = ew.tile([P, C], FP32, tag="y0")
                t1 = ew.tile([P, C], FP32, tag="t1")
                t2 = ew.tile([P, C], FP32, tag="t2")
                t3 = ew.tile([P, C], FP32, tag="t3")
                t4 = ew.tile([P, C], FP32, tag="t4")
                t5 = ew.tile([P, C], FP32, tag="t5")
                nc.vector.tensor_mul(t1[:], wz0[0], x0)
                nc.vector.tensor_mul(t2[:], wz0[1], x0sq)
                nc.vector.tensor_mul(t3[:], wz0[2], n1[:])
                nc.vector.tensor_mul(t4[:], wz0[3], x0cu[:])
                nc.vector.tensor_mul(t5[:], wz0[4], x0n1[:])
                nc.gpsimd.tensor_add(t1[:], t1[:], t2[:])
                nc.gpsimd.tensor_add(t3[:], t3[:], t4[:])
                nc.gpsimd.tensor_add(t1[:], t1[:], t5[:])
                nc.gpsimd.tensor_add(y0[:], t1[:], t3[:])

                # ---------------- c1 ----------------
                c1 = ew.tile([P, C], FP32, tag="c1")
                m1 = ew.tile([P, C], FP32, tag="m1")
                m2 = ew.tile([P, C], FP32, tag="m2")
                m3 = ew.tile([P, C], FP32, tag="m3")
                nc.vector.tensor_mul(m1[:], wz1[1], x0)
                nc.vector.tensor_mul(m2[:], wz1[2], x0sq)
                nc.vector.tensor_mul(m3[:], wz1[3], n1[:])
                nc.gpsimd.tensor_add(m1[:], m1[:], m2[:])
                nc.gpsimd.tensor_add(m1[:], m1[:], m3[:])
                nc.vector.tensor_add(c1[:], m1[:], wz1[0])

                # ---------------- y1m = c1 * x1m ----------------
                y1 = ew.tile([P, C, 3], FP32, tag="y1")
                c1ap = c1[:]
                c1b = bass.AP(tensor=c1ap.tensor, offset=c1ap.offset, ap=c1ap.ap + [[0, 3]])
                nc.vector.tensor_mul(y1[:], c1b, nfv[:, :, 1:4])

                # ---------------- TP products (pre-factored) ----------------
                s = ew.tile([P, C], FP32, tag="s")
                nc.vector.tensor_scalar_mul(s[:], y1[:, :, 0], a1[0])
                nc.vector.scalar_tensor_tensor(
                    s[:], y1[:, :, 1], a1[1], s[:], op0=OP.mult, op1=OP.add
                )
                nc.vector.scalar_tensor_tensor(
                    s[:], y1[:, :, 2], a1[2], s[:], op0=OP.mult, op1=OP.add
                )
                g2 = ew.tile([P, C], FP32, tag="g2")
                nc.vector.tensor_mul(g2[:], wb, s[:])

                g1a = ew.tile([P, C], FP32, tag="g1a")
                nc.vector.scalar_tensor_tensor(
                    g1a[:], wa, a0, y0[:], op0=OP.mult, op1=OP.mult
                )
                m1c = ew.tile([P, 3, C], FP32, tag="m1c")
                for m in range(3):
                    nc.vector.scalar_tensor_tensor(
                        m1c[:, m, :], wc, a1[m], y0[:], op0=OP.mult, op1=OP.mult
                    )
                rc = ew.tile([P, C], FP32, tag="rc")
                nc.vector.scalar_tensor_tensor(
                    rc[:], wd, a0, c1[:], op0=OP.mult, op1=OP.mult
                )
                hm = ew.tile([P, C, 3], FP32, tag="hm")
                rcap = rc[:]
                rcb = bass.AP(tensor=rcap.tensor, offset=rcap.offset, ap=rcap.ap + [[0, 3]])
                nc.vector.tensor_mul(hm[:], rcb, nfv[:, :, 1:4])

                # ---------------- transpose products to [C, nodes] ----------------
                def to_T(src_ap, tag, eng):
                    ps = t_ps_pool.tile([P, P], FP32, tag="tps")
                    nc.tensor.transpose(ps[:], src_ap, ident[:])
                    dst = prodT.tile([P, P], FP32, tag=tag)
                    if eng == "v":
                        nc.vector.tensor_copy(dst[:], ps[:])
                    else:
                        nc.scalar.copy(dst[:], ps[:])
                    return dst

                g1aT = to_T(g1a[:], "g1aT", "v")
                g2T = to_T(g2[:], "g2T", "s")
                y0T = to_T(y0[:], "y0T", "v")
                m1cT = [to_T(m1c[:, m, :], f"m1cT{m}", "s") for m in range(3)]
                hmT = [to_T(hm[:, :, m], f"hmT{m}", "v") for m in range(3)]
                y1T = [to_T(y1[:, :, m], f"y1T{m}", "s") for m in range(3)]

                # ---------------- output linear layers (C-layout) ----------------
                ops = o_ps_pool.tile([P, 4, P], FP32, tag="ops")
                nc.tensor.matmul(ops[:, 0, :], WA0, g1aT[:], start=True, stop=False)
                nc.tensor.matmul(ops[:, 0, :], WB0, g2T[:], start=False, stop=False)
                nc.tensor.matmul(ops[:, 0, :], WO0, y0T[:], start=False, stop=True)
                for m in range(3):
                    nc.tensor.matmul(
                        ops[:, 1 + m, :], WA1, m1cT[m][:], start=True, stop=False
                    )
                    nc.tensor.matmul(
                        ops[:, 1 + m, :], WB1, hmT[m][:], start=False, stop=False
                    )
                    nc.tensor.matmul(
                        ops[:, 1 + m, :], WO1, y1T[m][:], start=False, stop=True
                    )

                # ---------------- transpose back + add sc + store ----------------
                oT = prodT.tile([P, 4, P], FP32, tag="oT")
                for m in range(4):
                    if m % 2 == 0:
                        nc.vector.tensor_copy(oT[:, m, :], ops[:, m, :])
                    else:
                        nc.scalar.copy(oT[:, m, :], ops[:, m, :])
                nps0 = t_ps_pool.tile([P, P], FP32, tag="tps")
                nc.tensor.transpose(nps0[:], oT[:, 0, :], ident[:])
                nc.vector.tensor_add(out_t[:, 0:C], nps0[:], sc_t[:, 0:C])
                nps1 = o_ps_pool.tile([P, 3, P], FP32, tag="tps1")
                for m in range(3):
                    nc.tensor.transpose(nps1[:, m, :], oT[:, 1 + m, :], ident[:])
                # read psum (m,c)-major as (c,m) to match the interleaved layout
                npsv = nps1[:].rearrange("p m c -> p c m")
                nc.vector.tensor_add(out_t[:, C:], npsv, sc_t[:, C:])
            nc.sync.dma_start(out=out_r[:, s_, :], in_=out_st[:])

    nc.compile()
    return nc


_CACHE = {}


def _get_program(n_tiles):
    if n_tiles not in _CACHE:
        _CACHE[n_tiles] = build_program(n_tiles)
    return _CACHE[n_tiles]


def _in_map_for_core(inputs, c, n_core):
    lo, hi = c * n_core, (c + 1) * n_core
    return {
        "node_feats": np.ascontiguousarray(
            inputs["node_feats"][lo:hi].reshape(n_core, 4 * C)
        ),
        "sc": np.ascontiguousarray(inputs["sc"][lo:hi]),
        "node_attrs": np.ascontiguousarray(inputs["node_attrs"][lo:hi]),
        "magmom_node_inv_feats": np.ascontiguousarray(
            inputs["magmom_node_inv_feats"][lo:hi]
        ),
        "magmom_node_attrs": np.ascontiguousarray(inputs["magmom_node_attrs"][lo:hi]),
        "w_sc0": np.ascontiguousarray(inputs["w_sc0"].reshape(E, 5 * C)),
        "w_sc1": np.ascontiguousarray(inputs["w_sc1"].reshape(E, 4 * C)),
        "w_mlp1": np.asarray(inputs["w_mlp1"]),
        "w_mlp2": np.asarray(inputs["w_mlp2"]),
        "w_mlp3": np.asarray(inputs["w_mlp3"]),
        "w_mlp4": np.asarray(inputs["w_mlp4"]),
        "W_l0": np.asarray(inputs["W_l0"]),
        "W_l1": np.asarray(inputs["W_l1"]),
        "Wo0": np.asarray(inputs["Wo0"]),
        "Wo1": np.asarray(inputs["Wo1"]),
    }


def run_on_hw(inputs, trace=False):
    inputs = {k: np.asarray(v, dtype=np.float32) for k, v in inputs.items()}
    n_nodes = inputs["node_feats"].shape[0]
    n_core = n_nodes // N_CORES
    nc = _get_program(n_core // P)
    in_maps = [_in_map_for_core(inputs, c, n_core) for c in range(N_CORES)]
    res = run_bass_kernel_spmd(
        nc, in_maps, core_ids=list(range(N_CORES)), trace=trace
    )
    out = np.concatenate([res.results[c]["out"] for c in range(N_CORES)], axis=0)
    return out.astype(np.float32), res


def kernel(**inputs) -> np.ndarray:
    import os, time

    os.environ.setdefault("NEURON_RT_RESET_CORES", "1")
    try:
        out, _ = run_on_hw(inputs, trace=False)
    except Exception:
        time.sleep(5)
        out, _ = run_on_hw(inputs, trace=False)
    return out


def bench(inputs, iters=5):
    """Pipelined timing of the sharded NEFF execution (device-resident inputs)."""
    import time
    import jax
    from jax.sharding import Mesh, PartitionSpec
    from jax.experimental.shard_map import shard_map
    from concourse import bass2jax
    from concourse.bass2jax import _bass_exec_p, install_neuronx_cc_hook

    inputs = {k: np.asarray(v, dtype=np.float32) for k, v in inputs.items()}
    n_nodes = inputs["node_feats"].shape[0]
    n_core = n_nodes // N_CORES
    nc = _get_program(n_core // P)
    in_maps = [_in_map_for_core(inputs, c, n_core) for c in range(N_CORES)]

    install_neuronx_cc_hook()
    partition_name = nc.partition_id_tensor.name if nc.partition_id_tensor else None
    in_names, out_names, out_avals, zero_outs = [], [], [], []
    for alloc in nc.m.functions[0].allocations:
        if not isinstance(alloc, mybir.MemoryLocationSet):
            continue
        name = alloc.memorylocations[0].name
        if alloc.kind == "ExternalInput":
            if name != partition_name:
                in_names.append(name)
        elif alloc.kind == "ExternalOutput":
            out_names.append(name)
            shape = tuple(alloc.tensor_shape)
            dtype = mybir.dt.np(alloc.dtype)
            out_avals.append(jax.core.ShapedArray(shape, dtype))
            zero_outs.append(np.zeros(shape, dtype))
    n_params = len(in_names)
    all_names = in_names + out_names
    if partition_name is not None:
        all_names.append(partition_name)

    def _body(*args):
        operands = list(args)
        if partition_name is not None:
            operands.append(bass2jax.partition_id_tensor())
        return tuple(
            _bass_exec_p.bind(
                *operands,
                out_avals=tuple(out_avals),
                in_names=tuple(all_names),
                out_names=tuple(out_names),
                lowering_input_output_aliases=(),
                sim_require_finite=True,
                sim_require_nnan=True,
                nc=nc,
            )
        )

    devices = jax.devices()[:N_CORES]
    mesh = Mesh(np.asarray(devices), ("core",))
    nin = n_params + len(out_names)
    sharded = jax.jit(
        shard_map(
            _body,
            mesh=mesh,
            in_specs=(PartitionSpec("core"),) * nin,
            out_specs=(PartitionSpec("core"),) * len(out_names),
            check_rep=False,
        ),
        keep_unused=True,
    )
    per_core = [[np.asarray(m[nm]) for nm in in_names] for m in in_maps]
    concat_in = [
        np.concatenate([per_core[c][i] for c in range(N_CORES)], axis=0)
        for i in range(n_params)
    ]
    concat_zeros = [
        np.zeros((N_CORES * z.shape[0], *z.shape[1:]), z.dtype) for z in zero_outs
    ]
    from jax.sharding import NamedSharding
    sh = NamedSharding(mesh, PartitionSpec("core"))
    dev_in = [jax.device_put(a, sh) for a in concat_in + concat_zeros]
    out = sharded(*dev_in)
    jax.block_until_ready(out)
    t0 = time.time()
    for _ in range(iters):
        out = sharded(*dev_in)
    jax.block_until_ready(out)
    dt = (time.time() - t0) / iters
    return dt * 1e9, out



# revision 24
# speedup vs baseline: 1.0218x; 1.0218x over previous
"""Trainium2 Bass kernel for nn_EquivariantProductBasisWithSelfMagmomBlock.

Data-parallel over nodes: 8 NeuronCores x 8192 nodes each.

Channel-major design: per 512-node supertile, PE transposes the node-major
inputs into channel-major [c, n] tiles, all elementwise math runs on fp16
[128, 512] tiles (DVE 4x mode), matmul path weights / MLP / output linears
run as fp16 matmuls with fp32 PSUM accumulation.  The output linear uses the
channel-major mid tensors directly as matmul stationaries, producing
node-major output in PSUM (no back-transposes); a0/a1-scaled mid tensors
carry a 1/16 factor (folded into the broadcast) paired with 16x-scaled
output weights to keep fp16 products in range.

Node map inside a core: local node n = s*512 + q*128 + p.
"""

import sys

sys.path.insert(0, "/opt/trn_rl_repo")

from contextlib import ExitStack

import numpy as np

import concourse.bass as bass
import concourse.tile as tile
from concourse import bacc, mybir
from concourse.bass_utils import run_bass_kernel_spmd
from concourse.masks import make_identity

FP32 = mybir.dt.float32
F32R = mybir.dt.float32r
FP16 = mybir.dt.float16
AF = mybir.ActivationFunctionType
OP = mybir.AluOpType

N = 65536
C = 128
E = 10
INV = 16
N_CORES = 8
N_CORE = N // N_CORES  # 8192
P = 128
G = 512  # nodes per supertile

SCL = 16.0  # fp16 range guard: A-tiles carry 1/SCL, W_l* weights carry SCL


def r(ap):
    """bitcast an AP to float32r for full-rate fp32 matmul/transpose."""
    return ap.bitcast(F32R)


def build_program(n_tiles, use_silu=True):
    """Build the per-core SPMD program. n_tiles tiles of 128 nodes each.

    use_silu=False swaps Act-fused silu for sigmoid+DVE-mul (CoreSim lacks
    a Silu implementation; hardware has it in the silu_and_others table).
    """
    nc = bacc.Bacc(
        "TRN2", target_bir_lowering=False, debug=False, num_devices=N_CORES
    )
    n_nodes = n_tiles * P
    assert n_tiles % 4 == 0
    n_st = n_tiles // 4

    def din(name, shape):
        return nc.dram_tensor(name, list(shape), FP32, kind="ExternalInput").ap()

    nf_d = din("node_feats", (n_nodes, 4 * C))
    sc_d = din("sc", (n_nodes, 4 * C))
    attrs_d = din("node_attrs", (n_nodes, E))
    inv_d = din("magmom_node_inv_feats", (n_nodes, INV))
    mag_d = din("magmom_node_attrs", (n_nodes, 4))
    wsc0_d = din("w_sc0", (E, 5 * C))
    wsc1_d = din("w_sc1", (E, 4 * C))
    w1_d = din("w_mlp1", (INV, 64))
    w2_d = din("w_mlp2", (64, 64))
    w3_d = din("w_mlp3", (64, 64))
    w4_d = din("w_mlp4", (64, 4 * C))
    wl0_d = din("W_l0", (2 * C, C))
    wl1_d = din("W_l1", (2 * C, C))
    wo0_d = din("Wo0", (C, C))
    wo1_d = din("Wo1", (C, C))
    out_d = nc.dram_tensor("out", [n_nodes, 4 * C], FP32, kind="ExternalOutput").ap()

    # node n = s*512 + q*128 + p
    nf_r = nf_d.rearrange("(s q p) x -> p s q x", p=P, q=4)
    sc_r = sc_d.rearrange("(s q p) x -> p s q x", p=P, q=4)
    out_r = out_d.rearrange("(s q p) x -> p s q x", p=P, q=4)
    inv_r = inv_d.rearrange("(s q p) x -> p s q x", p=P, q=4)
    attrs_r = attrs_d.rearrange("(s q p) x -> p s q x", p=P, q=4)
    mag_r = mag_d.rearrange("(s q p) x -> p s q x", p=P, q=4)

    with tile.TileContext(nc) as tc, ExitStack() as ctx:
        singles = ctx.enter_context(tc.tile_pool(name="singles", bufs=1))
        nat = ctx.enter_context(tc.tile_pool(name="nat", bufs=2))
        ew = ctx.enter_context(tc.tile_pool(name="ew", bufs=2))
        # PSUM pools (8 banks): xps 1 + wz (zs 2 + zb 1) + work 2 + out 2 = 8
        xps_pool = ctx.enter_context(tc.tile_pool(name="xps", bufs=1, space="PSUM"))
        wz_pool = ctx.enter_context(tc.tile_pool(name="wzp", bufs=2, space="PSUM"))
        work_pool = ctx.enter_context(tc.tile_pool(name="work", bufs=2, space="PSUM"))
        out_pool = ctx.enter_context(tc.tile_pool(name="outp", bufs=2, space="PSUM"))

        # ---------------- preloads ----------------
        ident = singles.tile([P, P], FP32)
        make_identity(nc, ident[:])

        attrs_all = singles.tile([P, n_st, 4, E], FP32)
        nc.sync.dma_start(out=attrs_all[:], in_=attrs_r)
        inv_all = singles.tile([P, n_st, 4, INV], FP32)
        nc.sync.dma_start(out=inv_all[:], in_=inv_r)
        mag_all = singles.tile([P, n_st, 4, 4], FP32)
        nc.sync.dma_start(out=mag_all[:], in_=mag_r)

        # mag rows (a0, a1y, a1z) pre-transposed via strided DMA at
        # matmul-aligned partitions, converted once to fp16 rows.
        magT_f = singles.tile([65, n_nodes], FP32)
        nc.sync.dma_start(out=magT_f[0:1, :], in_=mag_d[:, 0:1].rearrange("n m -> m n"))
        nc.sync.dma_start(out=magT_f[32:33, :], in_=mag_d[:, 2:3].rearrange("n m -> m n"))
        nc.sync.dma_start(out=magT_f[64:65, :], in_=mag_d[:, 3:4].rearrange("n m -> m n"))
        magT_h = singles.tile([65, n_nodes], FP16)
        nc.scalar.copy(magT_h[0:1, :], magT_f[0:1, :])
        nc.scalar.copy(magT_h[32:33, :], magT_f[32:33, :])
        nc.scalar.copy(magT_h[64:65, :], magT_f[64:65, :])

        wscf = singles.tile([E, 9 * C], FP32)
        nc.sync.dma_start(out=wscf[:, 0 : 5 * C], in_=wsc0_d)
        nc.sync.dma_start(out=wscf[:, 5 * C : 9 * C], in_=wsc1_d)
        wsc_h = singles.tile([E, 9 * C], FP16)
        nc.vector.tensor_copy(wsc_h[:], wscf[:])

        w1f = singles.tile([INV, 64], FP32)
        nc.sync.dma_start(out=w1f[:], in_=w1_d)
        w2f = singles.tile([64, 64], FP32)
        nc.sync.dma_start(out=w2f[:], in_=w2_d)
        w3f = singles.tile([64, 64], FP32)
        nc.sync.dma_start(out=w3f[:], in_=w3_d)
        w4f = singles.tile([64, 4 * C], FP32)
        nc.sync.dma_start(out=w4f[:], in_=w4_d)
        w1h = singles.tile([INV, 64], FP16)
        nc.vector.tensor_copy(w1h[:], w1f[:])
        w2h = singles.tile([64, 64], FP16)
        nc.vector.tensor_copy(w2h[:], w2f[:])
        w3h = singles.tile([64, 64], FP16)
        nc.vector.tensor_copy(w3h[:], w3f[:])
        w4h = singles.tile([64, 4 * C], FP16)
        nc.vector.tensor_copy(w4h[:], w4f[:])

        # output weights: 0=WA0*S 1=WB0*S 2=WA1*S 3=WB1*S 4=Wo0 5=Wo1
        Wf = singles.tile([P, 6, C], FP32)
        nc.sync.dma_start(out=Wf[:, 0, :], in_=wl0_d[0:128, :])
        nc.sync.dma_start(out=Wf[:, 1, :], in_=wl0_d[128:256, :])
        nc.sync.dma_start(out=Wf[:, 2, :], in_=wl1_d[0:128, :])
        nc.sync.dma_start(out=Wf[:, 3, :], in_=wl1_d[128:256, :])
        nc.sync.dma_start(out=Wf[:, 4, :], in_=wo0_d)
        nc.sync.dma_start(out=Wf[:, 5, :], in_=wo1_d)
        Wh = singles.tile([P, 6, C], FP16)
        nc.scalar.activation(Wh[:, 0:4, :], Wf[:, 0:4, :], AF.Copy, scale=SCL)
        nc.scalar.copy(Wh[:, 4:6, :], Wf[:, 4:6, :])

        ones_t = singles.tile([65, P], FP16)
        nc.vector.memset(ones_t[:], 1.0 / SCL)

        for s_ in range(n_st):
            sl = slice(s_ * G, (s_ + 1) * G)
            # ---------------- supertile loads ----------------
            nf_st = nat.tile([P, 16 * C], FP32, tag="nf")
            nc.sync.dma_start(out=nf_st[:].rearrange("p (q x) -> p q x", q=4), in_=nf_r[:, s_])
            # sc loads directly into the output staging tile; final adds in-place
            out_st = nat.tile([P, 16 * C], FP32, tag="out")
            nc.sync.dma_start(out=out_st[:].rearrange("p (q x) -> p q x", q=4), in_=sc_r[:, s_])

            nfv = nf_st[:].rearrange("p (q c j) -> p q c j", q=4, j=4)

            # ------- attrs / inv / a1x transposes (partition-0 psum tiles) -------
            smA = work_pool.tile([E, G], FP32, tag="w")
            smI = work_pool.tile([INV, G], FP32, tag="w")
            smX = work_pool.tile([1, G], FP32, tag="w")
            for q in range(4):
                qs = slice(q * P, (q + 1) * P)
                nc.tensor.transpose(smA[:, qs], attrs_all[:, s_, q, :], ident[:])
                nc.tensor.transpose(smI[:, qs], inv_all[:, s_, q, :], ident[:])
                nc.tensor.transpose(smX[:, qs], mag_all[:, s_, q, 1:2], ident[:])
            aT = ew.tile([E, G], FP16, tag="aT")
            nc.vector.tensor_copy(aT[:], smA[:])
            iT = ew.tile([INV, G], FP16, tag="iT")
            nc.vector.tensor_copy(iT[:], smI[:])
            a1xh = ew.tile([1, G], FP16, tag="a1xh")
            nc.vector.tensor_copy(a1xh[:], smX[:])

            # ------- x transposes to channel-major + fp16 copies (Act) -------
            xh = ew.tile([P, 4, G], FP16, tag="xh")
            for comp in range(4):
                xp = xps_pool.tile([P, G], FP32, tag="x")
                for q in range(4):
                    nc.tensor.transpose(
                        xp[:, q * P : (q + 1) * P], nfv[:, q, :, comp], ident[:]
                    )
                nc.scalar.copy(xh[:, comp, :], xp[:])
            x0 = xh[:, 0, :]
            x1sl = xh[:, 1:4, :]

            # squares of all 4 components in one Act op; n1 = |x1|^2 (DVE)
            sq = ew.tile([P, 4, G], FP16, tag="sq")
            nc.scalar.activation(sq[:], xh[:], AF.Square)
            sq0 = sq[:, 0, :]
            n1t = ew.tile([P, G], FP16, tag="n1")
            nc.vector.tensor_add(n1t[:], sq[:, 1, :], sq[:, 2, :])
            nc.vector.tensor_add(n1t[:], n1t[:], sq[:, 3, :])

            # ------- A broadcasts (PE ones-matmul, carries 1/SCL; Act copies) ----
            A1 = ew.tile([P, 3, G], FP16, tag="A1")
            bsrc = [a1xh[0:1, :], magT_h[32:33, sl], magT_h[64:65, sl]]
            bbase = [0, 32, 64]
            for m in range(3):
                bp = work_pool.tile([P, G], FP32, tag="w")
                b0 = bbase[m]
                nc.tensor.matmul(bp[:], ones_t[b0 : b0 + 1, :], bsrc[m])
                nc.scalar.copy(A1[:, m, :], bp[:])
            A0h = ew.tile([64, G], FP16, tag="A0h")
            bp = work_pool.tile([P, G], FP32, tag="w")
            nc.tensor.matmul(bp[0:64, :], ones_t[0:1, 0:64], magT_h[0:1, sl])
            nc.scalar.copy(A0h[:], bp[0:64, :])

            # ------- wz paths; bases accumulate in PSUM via start=False mm -------
            def wz_mm(k, out=None, start=True, stop=True):
                if out is None:
                    out = wz_pool.tile([P, G], FP32, tag="zs")
                nc.tensor.matmul(
                    out[:], wsc_h[:, k * P : (k + 1) * P], aT[:],
                    start=start, stop=stop, skip_group_check=True,
                )
                return out

            # a = wz0 + wz1*x0 + wz3*sq0  (B accumulates wz1*x0 then +wz0)
            wp = wz_mm(1)
            Bb = wz_pool.tile([P, G], FP32, tag="zb", bufs=1)
            nc.vector.tensor_mul(Bb[:], wp[:], x0)
            wz_mm(0, out=Bb, start=False, stop=True)
            wp = wz_mm(3)
            t2 = ew.tile([P, G], FP16, tag="t2")
            nc.vector.tensor_mul(t2[:], wp[:], sq0)
            av = ew.tile([P, G], FP16, tag="av")
            nc.vector.tensor_add(av[:], Bb[:], t2[:])
            # c1 = wz5 + wz6*x0 + wz7*sq0 + wz8*n1
            wp = wz_mm(6)
            Cb = wz_pool.tile([P, G], FP32, tag="zb", bufs=1)
            nc.vector.tensor_mul(Cb[:], wp[:], x0)
            wz_mm(5, out=Cb, start=False, stop=True)
            wp = wz_mm(7)
            m2 = ew.tile([P, G], FP16, tag="m2")
            nc.vector.tensor_mul(m2[:], wp[:], sq0)
            wp = wz_mm(8)
            m3 = ew.tile([P, G], FP16, tag="m3")
            nc.vector.tensor_mul(m3[:], wp[:], n1t[:])
            c1 = ew.tile([P, G], FP16, tag="c1")
            nc.vector.tensor_add(c1[:], Cb[:], m2[:])
            nc.vector.tensor_add(c1[:], c1[:], m3[:])
            # b = wz2 + wz4*x0 ; y0 = x0*a + n1*b
            wp = wz_mm(4)
            Db = wz_pool.tile([P, G], FP32, tag="zb", bufs=1)
            nc.vector.tensor_mul(Db[:], wp[:], x0)
            wz_mm(2, out=Db, start=False, stop=True)
            y0 = ew.tile([P, G], FP16, tag="y0")
            ya = ew.tile([P, G], FP16, tag="ya")
            nc.vector.tensor_mul(ya[:], x0, av[:])
            nc.vector.tensor_mul(y0[:], n1t[:], Db[:])
            nc.vector.tensor_add(y0[:], y0[:], ya[:])

            # y1m = c1 * x1m  (Pool, batched via stride-0 rep of c1)
            y1t = ew.tile([P, 3, G], FP16, tag="y1t")
            c1ap = c1[:]
            c1b = bass.AP(
                tensor=c1ap.tensor, offset=c1ap.offset,
                ap=[c1ap.ap[0], [0, 3], c1ap.ap[1]],
            )
            nc.gpsimd.tensor_mul(y1t[:], c1b, x1sl)

            # s = sum_m y1m * A1m  (carries 1/SCL; Pool mul, DVE adds)
            smul = ew.tile([P, 3, G], FP16, tag="smul")
            nc.gpsimd.tensor_mul(smul[:], y1t[:], A1[:])
            sv = ew.tile([P, G], FP16, tag="sv")
            nc.vector.tensor_add(sv[:], smul[:, 0, :], smul[:, 1, :])
            nc.vector.tensor_add(sv[:], sv[:], smul[:, 2, :])

            # ------- magmom MLP (channel-major) -------
            h = iT
            hw_ = [w1h, w2h, w3h]
            for li in range(3):
                hp = work_pool.tile([64, G], FP32, tag="w")
                nc.tensor.matmul(hp[:], hw_[li][:], h[:])
                hn = ew.tile([64, G], FP16, tag=f"h{li}")
                if use_silu:
                    nc.scalar.activation(hn[:], hp[:], AF.Silu)
                else:
                    sg = ew.tile([64, G], FP16, tag=f"sg{li}")
                    nc.scalar.activation(sg[:], hp[:], AF.Sigmoid)
                    nc.vector.tensor_mul(hn[:], hp[:], sg[:])
                h = hn
            # a0-scaled copy of h3 feeds the wa/wd matmuls (folds a0/SCL in)
            h3a = ew.tile([64, G], FP16, tag="h3a")
            nc.vector.tensor_mul(h3a[:], h[:], A0h[:])

            # tpw quarters: wa,wd use h3a (a0-scaled); wb,wc use h
            wp = work_pool.tile([P, G], FP32, tag="w")
            nc.tensor.matmul(wp[:], w4h[:, 0:P], h3a[:])
            mid0a = ew.tile([P, G], FP16, tag="mid0a")
            nc.vector.tensor_mul(mid0a[:], wp[:], y0[:])
            wp = work_pool.tile([P, G], FP32, tag="w")
            nc.tensor.matmul(wp[:], w4h[:, P : 2 * P], h[:])
            g2 = ew.tile([P, G], FP16, tag="g2")
            nc.vector.tensor_mul(g2[:], wp[:], sv[:])
            wp = work_pool.tile([P, G], FP32, tag="w")
            nc.tensor.matmul(wp[:], w4h[:, 2 * P : 3 * P], h[:])
            wcy0 = ew.tile([P, G], FP16, tag="wcy0")
            nc.vector.tensor_mul(wcy0[:], wp[:], y0[:])
            m1c = ew.tile([P, 3, G], FP16, tag="m1c", bufs=1)
            wcap = wcy0[:]
            wcb = bass.AP(
                tensor=wcap.tensor, offset=wcap.offset,
                ap=[wcap.ap[0], [0, 3], wcap.ap[1]],
            )
            nc.gpsimd.tensor_mul(m1c[:], wcb, A1[:])
            wp = work_pool.tile([P, G], FP32, tag="w")
            nc.tensor.matmul(wp[:], w4h[:, 3 * P : 4 * P], h3a[:])
            rc2 = ew.tile([P, G], FP16, tag="rc2")
            nc.vector.tensor_mul(rc2[:], wp[:], c1[:])
            hm = ew.tile([P, 3, G], FP16, tag="hm", bufs=1)
            rcap = rc2[:]
            rcb = bass.AP(
                tensor=rcap.tensor, offset=rcap.offset,
                ap=[rcap.ap[0], [0, 3], rcap.ap[1]],
            )
            nc.gpsimd.tensor_mul(hm[:], rcb, x1sl)

            # ------- output linears: node-major PSUM via mid-stationary -------
            outv = out_st[:].rearrange("p (q f) -> p q f", q=4)

            o0p = out_pool.tile([P, 4, P], FP32, tag="o")
            for q in range(4):
                qs = slice(q * P, (q + 1) * P)
                nc.tensor.matmul(o0p[:, q, :], mid0a[:, qs], Wh[:, 0, :], start=True, stop=False)
                nc.tensor.matmul(o0p[:, q, :], g2[:, qs], Wh[:, 1, :], start=False, stop=False)
                nc.tensor.matmul(o0p[:, q, :], y0[:, qs], Wh[:, 4, :], start=False, stop=True)
            ov0 = outv[:, :, 0:C]
            nc.vector.tensor_add(ov0, o0p[:], ov0)

            for m in range(3):
                o1p = out_pool.tile([P, 4, P], FP32, tag="o")
                for q in range(4):
                    qs = slice(q * P, (q + 1) * P)
                    nc.tensor.matmul(o1p[:, q, :], m1c[:, m, qs], Wh[:, 2, :], start=True, stop=False)
                    nc.tensor.matmul(o1p[:, q, :], hm[:, m, qs], Wh[:, 3, :], start=False, stop=False)
                    nc.tensor.matmul(o1p[:, q, :], y1t[:, m, qs], Wh[:, 5, :], start=False, stop=True)
                ovm = outv[:, :, C : 4 * C].rearrange("p q (c j) -> p q c j", j=3)[:, :, :, m]
                nc.vector.tensor_add(ovm, o1p[:], ovm)

            nc.gpsimd.dma_start(out=out_r[:, s_], in_=out_st[:].rearrange("p (q x) -> p q x", q=4))

    nc.compile()
    return nc


_CACHE = {}


def _get_program(n_tiles):
    if n_tiles not in _CACHE:
        import os
        _CACHE[n_tiles] = build_program(
            n_tiles, use_silu=os.environ.get("K_NO_SILU", "") != "1"
        )
    return _CACHE[n_tiles]


def _in_map_for_core(inputs, c, n_core):
    lo, hi = c * n_core, (c + 1) * n_core
    return {
        "node_feats": np.ascontiguousarray(
            inputs["node_feats"][lo:hi].reshape(n_core, 4 * C)
        ),
        "sc": np.ascontiguousarray(inputs["sc"][lo:hi]),
        "node_attrs": np.ascontiguousarray(inputs["node_attrs"][lo:hi]),
        "magmom_node_inv_feats": np.ascontiguousarray(
            inputs["magmom_node_inv_feats"][lo:hi]
        ),
        "magmom_node_attrs": np.ascontiguousarray(inputs["magmom_node_attrs"][lo:hi]),
        "w_sc0": np.ascontiguousarray(inputs["w_sc0"].reshape(E, 5 * C)),
        "w_sc1": np.ascontiguousarray(inputs["w_sc1"].reshape(E, 4 * C)),
        "w_mlp1": np.asarray(inputs["w_mlp1"]),
        "w_mlp2": np.asarray(inputs["w_mlp2"]),
        "w_mlp3": np.asarray(inputs["w_mlp3"]),
        "w_mlp4": np.asarray(inputs["w_mlp4"]),
        "W_l0": np.asarray(inputs["W_l0"]),
        "W_l1": np.asarray(inputs["W_l1"]),
        "Wo0": np.asarray(inputs["Wo0"]),
        "Wo1": np.asarray(inputs["Wo1"]),
    }


def run_on_hw(inputs, trace=False):
    inputs = {k: np.asarray(v, dtype=np.float32) for k, v in inputs.items()}
    n_nodes = inputs["node_feats"].shape[0]
    n_core = n_nodes // N_CORES
    nc = _get_program(n_core // P)
    in_maps = [_in_map_for_core(inputs, c, n_core) for c in range(N_CORES)]
    res = run_bass_kernel_spmd(
        nc, in_maps, core_ids=list(range(N_CORES)), trace=trace
    )
    out = np.concatenate([res.results[c]["out"] for c in range(N_CORES)], axis=0)
    return out.astype(np.float32), res


def kernel(**inputs) -> np.ndarray:
    import os, time

    os.environ.setdefault("NEURON_RT_RESET_CORES", "1")
    try:
        out, _ = run_on_hw(inputs, trace=False)
    except Exception:
        time.sleep(5)
        out, _ = run_on_hw(inputs, trace=False)
    return out


def bench(inputs, iters=5):
    """Pipelined timing of the sharded NEFF execution (device-resident inputs)."""
    import time
    import jax
    from jax.sharding import Mesh, PartitionSpec
    from jax.experimental.shard_map import shard_map
    from concourse import bass2jax
    from concourse.bass2jax import _bass_exec_p, install_neuronx_cc_hook

    inputs = {k: np.asarray(v, dtype=np.float32) for k, v in inputs.items()}
    n_nodes = inputs["node_feats"].shape[0]
    n_core = n_nodes // N_CORES
    nc = _get_program(n_core // P)
    in_maps = [_in_map_for_core(inputs, c, n_core) for c in range(N_CORES)]

    install_neuronx_cc_hook()
    partition_name = nc.partition_id_tensor.name if nc.partition_id_tensor else None
    in_names, out_names, out_avals, zero_outs = [], [], [], []
    for alloc in nc.m.functions[0].allocations:
        if not isinstance(alloc, mybir.MemoryLocationSet):
            continue
        name = alloc.memorylocations[0].name
        if alloc.kind == "ExternalInput":
            if name != partition_name:
                in_names.append(name)
        elif alloc.kind == "ExternalOutput":
            out_names.append(name)
            shape = tuple(alloc.tensor_shape)
            dtype = mybir.dt.np(alloc.dtype)
            out_avals.append(jax.core.ShapedArray(shape, dtype))
            zero_outs.append(np.zeros(shape, dtype))
    n_params = len(in_names)
    all_names = in_names + out_names
    if partition_name is not None:
        all_names.append(partition_name)

    def _body(*args):
        operands = list(args)
        if partition_name is not None:
            operands.append(bass2jax.partition_id_tensor())
        return tuple(
            _bass_exec_p.bind(
                *operands,
                out_avals=tuple(out_avals),
                in_names=tuple(all_names),
                out_names=tuple(out_names),
                lowering_input_output_aliases=(),
                sim_require_finite=True,
                sim_require_nnan=True,
                nc=nc,
            )
        )

    devices = jax.devices()[:N_CORES]
    mesh = Mesh(np.asarray(devices), ("core",))
    nin = n_params + len(out_names)
    sharded = jax.jit(
        shard_map(
            _body,
            mesh=mesh,
            in_specs=(PartitionSpec("core"),) * nin,
            out_specs=(PartitionSpec("core"),) * len(out_names),
            check_rep=False,
        ),
        keep_unused=True,
    )
    per_core = [[np.asarray(m[nm]) for nm in in_names] for m in in_maps]
    concat_in = [
        np.concatenate([per_core[c][i] for c in range(N_CORES)], axis=0)
        for i in range(n_params)
    ]
    concat_zeros = [
        np.zeros((N_CORES * z.shape[0], *z.shape[1:]), z.dtype) for z in zero_outs
    ]
    from jax.sharding import NamedSharding
    sh = NamedSharding(mesh, PartitionSpec("core"))
    dev_in = [jax.device_put(a, sh) for a in concat_in + concat_zeros]
    out = sharded(*dev_in)
    jax.block_until_ready(out)
    t0 = time.time()
    for _ in range(iters):
        out = sharded(*dev_in)
    jax.block_until_ready(out)
    dt = (time.time() - t0) / iters
    return dt * 1e9, out


# revision 27
# speedup vs baseline: 1.0307x; 1.0087x over previous
"""Trainium2 Bass kernel for nn_EquivariantProductBasisWithSelfMagmomBlock.

Data-parallel over nodes: 8 NeuronCores x 8192 nodes each.

Channel-major design: per 512-node supertile, PE transposes the node-major
inputs into channel-major [c, n] tiles, all elementwise math runs on fp16
[128, 512] tiles (DVE 4x mode), matmul path weights / MLP / output linears
run as fp16 matmuls with fp32 PSUM accumulation.  The output linear uses the
channel-major mid tensors directly as matmul stationaries, producing
node-major output in PSUM (no back-transposes); a0/a1-scaled mid tensors
carry a 1/16 factor (folded into the broadcast) paired with 16x-scaled
output weights to keep fp16 products in range.

Node map inside a core: local node n = s*512 + q*128 + p.
"""

import sys

sys.path.insert(0, "/opt/trn_rl_repo")

from contextlib import ExitStack

import numpy as np

import concourse.bass as bass
import concourse.tile as tile
from concourse import bacc, mybir
from concourse.bass_utils import run_bass_kernel_spmd
from concourse.masks import make_identity

FP32 = mybir.dt.float32
F32R = mybir.dt.float32r
FP16 = mybir.dt.float16
AF = mybir.ActivationFunctionType
OP = mybir.AluOpType

N = 65536
C = 128
E = 10
INV = 16
N_CORES = 8
N_CORE = N // N_CORES  # 8192
P = 128
G = 512  # nodes per supertile

SCL = 16.0  # fp16 range guard: A-tiles carry 1/SCL, W_l* weights carry SCL


def r(ap):
    """bitcast an AP to float32r for full-rate fp32 matmul/transpose."""
    return ap.bitcast(F32R)


def build_program(n_tiles, use_silu=True):
    """Build the per-core SPMD program. n_tiles tiles of 128 nodes each.

    use_silu=False swaps Act-fused silu for sigmoid+DVE-mul (CoreSim lacks
    a Silu implementation; hardware has it in the silu_and_others table).
    """
    nc = bacc.Bacc(
        "TRN2", target_bir_lowering=False, debug=False, num_devices=N_CORES
    )
    n_nodes = n_tiles * P
    assert n_tiles % 4 == 0
    n_st = n_tiles // 4

    def din(name, shape):
        return nc.dram_tensor(name, list(shape), FP32, kind="ExternalInput").ap()

    nf_d = din("node_feats", (n_nodes, 4 * C))
    sc_d = din("sc", (n_nodes, 4 * C))
    attrs_d = din("node_attrs", (n_nodes, E))
    inv_d = din("magmom_node_inv_feats", (n_nodes, INV))
    mag_d = din("magmom_node_attrs", (n_nodes, 4))
    wsc0_d = din("w_sc0", (E, 5 * C))
    wsc1_d = din("w_sc1", (E, 4 * C))
    w1_d = din("w_mlp1", (INV, 64))
    w2_d = din("w_mlp2", (64, 64))
    w3_d = din("w_mlp3", (64, 64))
    w4_d = din("w_mlp4", (64, 4 * C))
    wl0_d = din("W_l0", (2 * C, C))
    wl1_d = din("W_l1", (2 * C, C))
    wo0_d = din("Wo0", (C, C))
    wo1_d = din("Wo1", (C, C))
    out_d = nc.dram_tensor("out", [n_nodes, 4 * C], FP32, kind="ExternalOutput").ap()

    # node n = s*512 + q*128 + p
    nf_r = nf_d.rearrange("(s q p) x -> p s q x", p=P, q=4)
    sc_r = sc_d.rearrange("(s q p) x -> p s q x", p=P, q=4)
    out_r = out_d.rearrange("(s q p) x -> p s q x", p=P, q=4)
    inv_r = inv_d.rearrange("(s q p) x -> p s q x", p=P, q=4)
    attrs_r = attrs_d.rearrange("(s q p) x -> p s q x", p=P, q=4)
    mag_r = mag_d.rearrange("(s q p) x -> p s q x", p=P, q=4)

    with tile.TileContext(nc) as tc, ExitStack() as ctx:
        singles = ctx.enter_context(tc.tile_pool(name="singles", bufs=1))
        nat = ctx.enter_context(tc.tile_pool(name="nat", bufs=2))
        ew = ctx.enter_context(tc.tile_pool(name="ew", bufs=2))
        # PSUM pools (8 banks): xps 1 + wz (zs 2 + zb 1) + work 2 + out 2 = 8
        xps_pool = ctx.enter_context(tc.tile_pool(name="xps", bufs=1, space="PSUM"))
        wz_pool = ctx.enter_context(tc.tile_pool(name="wzp", bufs=2, space="PSUM"))
        work_pool = ctx.enter_context(tc.tile_pool(name="work", bufs=2, space="PSUM"))
        out_pool = ctx.enter_context(tc.tile_pool(name="outp", bufs=2, space="PSUM"))

        # ---------------- preloads ----------------
        ident = singles.tile([P, P], FP32)
        make_identity(nc, ident[:])

        attrs_all = singles.tile([P, n_st, 4, E], FP32)
        nc.sync.dma_start(out=attrs_all[:], in_=attrs_r)
        inv_all = singles.tile([P, n_st, 4, INV], FP32)
        nc.sync.dma_start(out=inv_all[:], in_=inv_r)
        mag_all = singles.tile([P, n_st, 4, 4], FP32)
        nc.sync.dma_start(out=mag_all[:], in_=mag_r)

        # mag rows (a0, a1y, a1z) pre-transposed via strided DMA at
        # matmul-aligned partitions, converted once to fp16 rows.
        magT_f = singles.tile([65, n_nodes], FP32)
        nc.sync.dma_start(out=magT_f[0:1, :], in_=mag_d[:, 0:1].rearrange("n m -> m n"))
        nc.sync.dma_start(out=magT_f[32:33, :], in_=mag_d[:, 2:3].rearrange("n m -> m n"))
        nc.sync.dma_start(out=magT_f[64:65, :], in_=mag_d[:, 3:4].rearrange("n m -> m n"))
        magT_h = singles.tile([65, n_nodes], FP16)
        nc.scalar.copy(magT_h[0:1, :], magT_f[0:1, :])
        nc.scalar.copy(magT_h[32:33, :], magT_f[32:33, :])
        nc.scalar.copy(magT_h[64:65, :], magT_f[64:65, :])

        wscf = singles.tile([E, 9 * C], FP32)
        nc.sync.dma_start(out=wscf[:, 0 : 5 * C], in_=wsc0_d)
        nc.sync.dma_start(out=wscf[:, 5 * C : 9 * C], in_=wsc1_d)
        wsc_h = singles.tile([E, 9 * C], FP16)
        nc.vector.tensor_copy(wsc_h[:], wscf[:])

        w1f = singles.tile([INV, 64], FP32)
        nc.sync.dma_start(out=w1f[:], in_=w1_d)
        w2f = singles.tile([64, 64], FP32)
        nc.sync.dma_start(out=w2f[:], in_=w2_d)
        w3f = singles.tile([64, 64], FP32)
        nc.sync.dma_start(out=w3f[:], in_=w3_d)
        w4f = singles.tile([64, 4 * C], FP32)
        nc.sync.dma_start(out=w4f[:], in_=w4_d)
        w1h = singles.tile([INV, 64], FP16)
        nc.vector.tensor_copy(w1h[:], w1f[:])
        w2h = singles.tile([64, 64], FP16)
        nc.vector.tensor_copy(w2h[:], w2f[:])
        w3h = singles.tile([64, 64], FP16)
        nc.vector.tensor_copy(w3h[:], w3f[:])
        w4h = singles.tile([64, 4 * C], FP16)
        nc.vector.tensor_copy(w4h[:], w4f[:])

        # output weights: 0=WA0*S 1=WB0*S 2=WA1*S 3=WB1*S 4=Wo0 5=Wo1
        Wf = singles.tile([P, 6, C], FP32)
        nc.sync.dma_start(out=Wf[:, 0, :], in_=wl0_d[0:128, :])
        nc.sync.dma_start(out=Wf[:, 1, :], in_=wl0_d[128:256, :])
        nc.sync.dma_start(out=Wf[:, 2, :], in_=wl1_d[0:128, :])
        nc.sync.dma_start(out=Wf[:, 3, :], in_=wl1_d[128:256, :])
        nc.sync.dma_start(out=Wf[:, 4, :], in_=wo0_d)
        nc.sync.dma_start(out=Wf[:, 5, :], in_=wo1_d)
        Wh = singles.tile([P, 6, C], FP16)
        nc.scalar.activation(Wh[:, 0:4, :], Wf[:, 0:4, :], AF.Copy, scale=SCL)
        nc.scalar.copy(Wh[:, 4:6, :], Wf[:, 4:6, :])

        ones_t = singles.tile([65, P], FP16)
        nc.vector.memset(ones_t[:], 1.0 / SCL)

        for s_ in range(n_st):
            sl = slice(s_ * G, (s_ + 1) * G)
            # ---------------- supertile loads ----------------
            nf_st = nat.tile([P, 16 * C], FP32, tag="nf")
            nc.sync.dma_start(out=nf_st[:].rearrange("p (q x) -> p q x", q=4), in_=nf_r[:, s_])
            # sc loads directly into the output staging tile; final adds in-place
            out_st = nat.tile([P, 16 * C], FP32, tag="out")
            nc.sync.dma_start(out=out_st[:].rearrange("p (q x) -> p q x", q=4), in_=sc_r[:, s_])

            nfv = nf_st[:].rearrange("p (q c j) -> p q c j", q=4, j=4)

            # ------- attrs / inv / a1x transposes (partition-0 psum tiles) -------
            smA = work_pool.tile([E, G], FP32, tag="w")
            smI = work_pool.tile([INV, G], FP32, tag="w")
            smX = work_pool.tile([1, G], FP32, tag="w")
            for q in range(4):
                qs = slice(q * P, (q + 1) * P)
                nc.tensor.transpose(smA[:, qs], attrs_all[:, s_, q, :], ident[:])
                nc.tensor.transpose(smI[:, qs], inv_all[:, s_, q, :], ident[:])
                nc.tensor.transpose(smX[:, qs], mag_all[:, s_, q, 1:2], ident[:])
            aT = ew.tile([E, G], FP16, tag="aT")
            nc.vector.tensor_copy(aT[:], smA[:])
            iT = ew.tile([INV, G], FP16, tag="iT")
            nc.vector.tensor_copy(iT[:], smI[:])
            a1xh = ew.tile([1, G], FP16, tag="a1xh")
            nc.vector.tensor_copy(a1xh[:], smX[:])

            # ------- x transposes to channel-major + fp16 copies (Act) -------
            x0f = ew.tile([P, G], FP32, tag="x0f")
            xh = ew.tile([P, 3, G], FP16, tag="xh")
            for comp in range(4):
                xp = xps_pool.tile([P, G], FP32, tag="x")
                for q in range(4):
                    nc.tensor.transpose(
                        xp[:, q * P : (q + 1) * P], nfv[:, q, :, comp], ident[:]
                    )
                if comp == 0:
                    nc.scalar.copy(x0f[:], xp[:])
                else:
                    nc.scalar.copy(xh[:, comp - 1, :], xp[:])
            x0 = x0f[:]
            x1sl = xh[:, 0:3, :]

            # squares: x0^2 fp32 on Act, x1 squares fp16 ; n1 = |x1|^2 (fp32)
            sq0t = ew.tile([P, G], FP32, tag="sq0t")
            nc.scalar.activation(sq0t[:], x0f[:], AF.Square)
            sq0 = sq0t[:]
            sq1 = ew.tile([P, 3, G], FP16, tag="sq1", bufs=1)
            nc.scalar.activation(sq1[:], xh[:], AF.Square)
            n1t = ew.tile([P, G], FP32, tag="n1")
            nc.vector.tensor_add(n1t[:], sq1[:, 0, :], sq1[:, 1, :])
            nc.vector.tensor_add(n1t[:], n1t[:], sq1[:, 2, :])

            # ------- A broadcasts (PE ones-matmul, carries 1/SCL; Act copies) ----
            A1 = ew.tile([P, 3, G], FP16, tag="A1")
            bsrc = [a1xh[0:1, :], magT_h[32:33, sl], magT_h[64:65, sl]]
            bbase = [0, 32, 64]
            for m in range(3):
                bp = work_pool.tile([P, G], FP32, tag="w")
                b0 = bbase[m]
                nc.tensor.matmul(bp[:], ones_t[b0 : b0 + 1, :], bsrc[m])
                nc.scalar.copy(A1[:, m, :], bp[:])
            A0h = ew.tile([64, G], FP16, tag="A0h")
            bp = work_pool.tile([P, G], FP32, tag="w")
            nc.tensor.matmul(bp[0:64, :], ones_t[0:1, 0:64], magT_h[0:1, sl])
            nc.scalar.copy(A0h[:], bp[0:64, :])

            # ------- wz paths; bases accumulate in PSUM via start=False mm -------
            def wz_mm(k, out=None, start=True, stop=True):
                if out is None:
                    out = wz_pool.tile([P, G], FP32, tag="zs")
                nc.tensor.matmul(
                    out[:], wsc_h[:, k * P : (k + 1) * P], aT[:],
                    start=start, stop=stop, skip_group_check=True,
                )
                return out

            # a = wz0 + wz1*x0 + wz3*sq0  (B accumulates wz1*x0 then +wz0)
            wp = wz_mm(1)
            Bb = wz_pool.tile([P, G], FP32, tag="zb", bufs=1)
            nc.vector.tensor_mul(Bb[:], wp[:], x0)
            wz_mm(0, out=Bb, start=False, stop=True)
            wp = wz_mm(3)
            t2 = ew.tile([P, G], FP32, tag="t2", bufs=1)
            nc.vector.tensor_mul(t2[:], wp[:], sq0)
            av = ew.tile([P, G], FP32, tag="av")
            nc.vector.tensor_add(av[:], Bb[:], t2[:])
            # c1 = wz5 + wz6*x0 + wz7*sq0 + wz8*n1
            wp = wz_mm(6)
            Cb = wz_pool.tile([P, G], FP32, tag="zb", bufs=1)
            nc.vector.tensor_mul(Cb[:], wp[:], x0)
            wz_mm(5, out=Cb, start=False, stop=True)
            wp = wz_mm(7)
            m2 = ew.tile([P, G], FP32, tag="m2", bufs=1)
            nc.vector.tensor_mul(m2[:], wp[:], sq0)
            wp = wz_mm(8)
            m3 = ew.tile([P, G], FP32, tag="m3", bufs=1)
            nc.vector.tensor_mul(m3[:], wp[:], n1t[:])
            c1 = ew.tile([P, G], FP32, tag="c1")
            nc.vector.tensor_add(c1[:], Cb[:], m2[:])
            nc.vector.tensor_add(c1[:], c1[:], m3[:])
            # b = wz2 + wz4*x0 ; y0 = x0*a + n1*b
            wp = wz_mm(4)
            Db = wz_pool.tile([P, G], FP32, tag="zb", bufs=1)
            nc.vector.tensor_mul(Db[:], wp[:], x0)
            wz_mm(2, out=Db, start=False, stop=True)
            y0 = ew.tile([P, G], FP32, tag="y0")
            ya = ew.tile([P, G], FP32, tag="ya")
            nc.vector.tensor_mul(ya[:], x0, av[:])
            nc.vector.tensor_mul(y0[:], n1t[:], Db[:])
            nc.vector.tensor_add(y0[:], y0[:], ya[:])

            y0h = ew.tile([P, G], FP16, tag="y0h")
            nc.vector.tensor_copy(y0h[:], y0[:])
            c1h = ew.tile([P, G], FP16, tag="c1h")
            nc.vector.tensor_copy(c1h[:], c1[:])

            # y1m = c1 * x1m  (Pool, batched via stride-0 rep of c1)
            y1t = ew.tile([P, 3, G], FP16, tag="y1t")
            c1ap = c1h[:]
            c1b = bass.AP(
                tensor=c1ap.tensor, offset=c1ap.offset,
                ap=[c1ap.ap[0], [0, 3], c1ap.ap[1]],
            )
            nc.gpsimd.tensor_mul(y1t[:], c1b, x1sl)

            # s = sum_m y1m * A1m  (carries 1/SCL; Pool mul, DVE adds)
            smul = ew.tile([P, 3, G], FP16, tag="smul")
            nc.gpsimd.tensor_mul(smul[:], y1t[:], A1[:])
            sv = ew.tile([P, G], FP16, tag="sv")
            nc.vector.tensor_add(sv[:], smul[:, 0, :], smul[:, 1, :])
            nc.vector.tensor_add(sv[:], sv[:], smul[:, 2, :])

            # ------- magmom MLP (channel-major) -------
            h = iT
            hw_ = [w1h, w2h, w3h]
            for li in range(3):
                hp = work_pool.tile([64, G], FP32, tag="w")
                nc.tensor.matmul(hp[:], hw_[li][:], h[:])
                hn = ew.tile([64, G], FP16, tag=f"h{li}")
                sg = ew.tile([64, G], FP16, tag=f"sg{li}")
                nc.scalar.activation(sg[:], hp[:], AF.Sigmoid)
                nc.vector.tensor_mul(hn[:], hp[:], sg[:])
                h = hn
            # a0-scaled copy of h3 feeds the wa/wd matmuls (folds a0/SCL in)
            h3a = ew.tile([64, G], FP16, tag="h3a")
            nc.vector.tensor_mul(h3a[:], h[:], A0h[:])

            # tpw quarters: wa,wd use h3a (a0-scaled); wb,wc use h
            wp = work_pool.tile([P, G], FP32, tag="w")
            nc.tensor.matmul(wp[:], w4h[:, 0:P], h3a[:])
            mid0a = ew.tile([P, G], FP16, tag="mid0a")
            nc.vector.tensor_mul(mid0a[:], wp[:], y0[:])
            wp = work_pool.tile([P, G], FP32, tag="w")
            nc.tensor.matmul(wp[:], w4h[:, P : 2 * P], h[:])
            g2 = ew.tile([P, G], FP16, tag="g2")
            nc.vector.tensor_mul(g2[:], wp[:], sv[:])
            wp = work_pool.tile([P, G], FP32, tag="w")
            nc.tensor.matmul(wp[:], w4h[:, 2 * P : 3 * P], h[:])
            wcy0 = ew.tile([P, G], FP16, tag="wcy0")
            nc.vector.tensor_mul(wcy0[:], wp[:], y0[:])
            m1c = ew.tile([P, 3, G], FP16, tag="m1c", bufs=1)
            wcap = wcy0[:]
            wcb = bass.AP(
                tensor=wcap.tensor, offset=wcap.offset,
                ap=[wcap.ap[0], [0, 3], wcap.ap[1]],
            )
            nc.gpsimd.tensor_mul(m1c[:], wcb, A1[:])
            wp = work_pool.tile([P, G], FP32, tag="w")
            nc.tensor.matmul(wp[:], w4h[:, 3 * P : 4 * P], h3a[:])
            rc2 = ew.tile([P, G], FP16, tag="rc2")
            nc.vector.tensor_mul(rc2[:], wp[:], c1[:])
            hm = ew.tile([P, 3, G], FP16, tag="hm", bufs=1)
            rcap = rc2[:]
            rcb = bass.AP(
                tensor=rcap.tensor, offset=rcap.offset,
                ap=[rcap.ap[0], [0, 3], rcap.ap[1]],
            )
            nc.gpsimd.tensor_mul(hm[:], rcb, x1sl)

            # ------- output linears: node-major PSUM via mid-stationary -------
            outv = out_st[:].rearrange("p (q f) -> p q f", q=4)

            o0p = out_pool.tile([P, 4, P], FP32, tag="o")
            for q in range(4):
                qs = slice(q * P, (q + 1) * P)
                nc.tensor.matmul(o0p[:, q, :], mid0a[:, qs], Wh[:, 0, :], start=True, stop=False)
                nc.tensor.matmul(o0p[:, q, :], g2[:, qs], Wh[:, 1, :], start=False, stop=False)
                nc.tensor.matmul(o0p[:, q, :], y0h[:, qs], Wh[:, 4, :], start=False, stop=True)
            ov0 = outv[:, :, 0:C]
            nc.vector.tensor_add(ov0, o0p[:], ov0)

            for m in range(3):
                o1p = out_pool.tile([P, 4, P], FP32, tag="o")
                for q in range(4):
                    qs = slice(q * P, (q + 1) * P)
                    nc.tensor.matmul(o1p[:, q, :], m1c[:, m, qs], Wh[:, 2, :], start=True, stop=False)
                    nc.tensor.matmul(o1p[:, q, :], hm[:, m, qs], Wh[:, 3, :], start=False, stop=False)
                    nc.tensor.matmul(o1p[:, q, :], y1t[:, m, qs], Wh[:, 5, :], start=False, stop=True)
                ovm = outv[:, :, C : 4 * C].rearrange("p q (c j) -> p q c j", j=3)[:, :, :, m]
                nc.vector.tensor_add(ovm, o1p[:], ovm)

            nc.gpsimd.dma_start(out=out_r[:, s_], in_=out_st[:].rearrange("p (q x) -> p q x", q=4))

    nc.compile()
    return nc


_CACHE = {}


def _get_program(n_tiles):
    if n_tiles not in _CACHE:
        import os
        _CACHE[n_tiles] = build_program(
            n_tiles, use_silu=os.environ.get("K_NO_SILU", "") != "1"
        )
    return _CACHE[n_tiles]


def _in_map_for_core(inputs, c, n_core):
    lo, hi = c * n_core, (c + 1) * n_core
    return {
        "node_feats": np.ascontiguousarray(
            inputs["node_feats"][lo:hi].reshape(n_core, 4 * C)
        ),
        "sc": np.ascontiguousarray(inputs["sc"][lo:hi]),
        "node_attrs": np.ascontiguousarray(inputs["node_attrs"][lo:hi]),
        "magmom_node_inv_feats": np.ascontiguousarray(
            inputs["magmom_node_inv_feats"][lo:hi]
        ),
        "magmom_node_attrs": np.ascontiguousarray(inputs["magmom_node_attrs"][lo:hi]),
        "w_sc0": np.ascontiguousarray(inputs["w_sc0"].reshape(E, 5 * C)),
        "w_sc1": np.ascontiguousarray(inputs["w_sc1"].reshape(E, 4 * C)),
        "w_mlp1": np.asarray(inputs["w_mlp1"]),
        "w_mlp2": np.asarray(inputs["w_mlp2"]),
        "w_mlp3": np.asarray(inputs["w_mlp3"]),
        "w_mlp4": np.asarray(inputs["w_mlp4"]),
        "W_l0": np.asarray(inputs["W_l0"]),
        "W_l1": np.asarray(inputs["W_l1"]),
        "Wo0": np.asarray(inputs["Wo0"]),
        "Wo1": np.asarray(inputs["Wo1"]),
    }


def run_on_hw(inputs, trace=False):
    inputs = {k: np.asarray(v, dtype=np.float32) for k, v in inputs.items()}
    n_nodes = inputs["node_feats"].shape[0]
    n_core = n_nodes // N_CORES
    nc = _get_program(n_core // P)
    in_maps = [_in_map_for_core(inputs, c, n_core) for c in range(N_CORES)]
    res = run_bass_kernel_spmd(
        nc, in_maps, core_ids=list(range(N_CORES)), trace=trace
    )
    out = np.concatenate([res.results[c]["out"] for c in range(N_CORES)], axis=0)
    return out.astype(np.float32), res


def kernel(**inputs) -> np.ndarray:
    import os, time

    os.environ.setdefault("NEURON_RT_RESET_CORES", "1")
    try:
        out, _ = run_on_hw(inputs, trace=False)
    except Exception:
        time.sleep(5)
        out, _ = run_on_hw(inputs, trace=False)
    return out


def bench(inputs, iters=5):
    """Pipelined timing of the sharded NEFF execution (device-resident inputs)."""
    import time
    import jax
    from jax.sharding import Mesh, PartitionSpec
    from jax.experimental.shard_map import shard_map
    from concourse import bass2jax
    from concourse.bass2jax import _bass_exec_p, install_neuronx_cc_hook

    inputs = {k: np.asarray(v, dtype=np.float32) for k, v in inputs.items()}
    n_nodes = inputs["node_feats"].shape[0]
    n_core = n_nodes // N_CORES
    nc = _get_program(n_core // P)
    in_maps = [_in_map_for_core(inputs, c, n_core) for c in range(N_CORES)]

    install_neuronx_cc_hook()
    partition_name = nc.partition_id_tensor.name if nc.partition_id_tensor else None
    in_names, out_names, out_avals, zero_outs = [], [], [], []
    for alloc in nc.m.functions[0].allocations:
        if not isinstance(alloc, mybir.MemoryLocationSet):
            continue
        name = alloc.memorylocations[0].name
        if alloc.kind == "ExternalInput":
            if name != partition_name:
                in_names.append(name)
        elif alloc.kind == "ExternalOutput":
            out_names.append(name)
            shape = tuple(alloc.tensor_shape)
            dtype = mybir.dt.np(alloc.dtype)
            out_avals.append(jax.core.ShapedArray(shape, dtype))
            zero_outs.append(np.zeros(shape, dtype))
    n_params = len(in_names)
    all_names = in_names + out_names
    if partition_name is not None:
        all_names.append(partition_name)

    def _body(*args):
        operands = list(args)
        if partition_name is not None:
            operands.append(bass2jax.partition_id_tensor())
        return tuple(
            _bass_exec_p.bind(
                *operands,
                out_avals=tuple(out_avals),
                in_names=tuple(all_names),
                out_names=tuple(out_names),
                lowering_input_output_aliases=(),
                sim_require_finite=True,
                sim_require_nnan=True,
                nc=nc,
            )
        )

    devices = jax.devices()[:N_CORES]
    mesh = Mesh(np.asarray(devices), ("core",))
    nin = n_params + len(out_names)
    sharded = jax.jit(
        shard_map(
            _body,
            mesh=mesh,
            in_specs=(PartitionSpec("core"),) * nin,
            out_specs=(PartitionSpec("core"),) * len(out_names),
            check_rep=False,
        ),
        keep_unused=True,
    )
    per_core = [[np.asarray(m[nm]) for nm in in_names] for m in in_maps]
    concat_in = [
        np.concatenate([per_core[c][i] for c in range(N_CORES)], axis=0)
        for i in range(n_params)
    ]
    concat_zeros = [
        np.zeros((N_CORES * z.shape[0], *z.shape[1:]), z.dtype) for z in zero_outs
    ]
    from jax.sharding import NamedSharding
    sh = NamedSharding(mesh, PartitionSpec("core"))
    dev_in = [jax.device_put(a, sh) for a in concat_in + concat_zeros]
    out = sharded(*dev_in)
    jax.block_until_ready(out)
    t0 = time.time()
    for _ in range(iters):
        out = sharded(*dev_in)
    jax.block_until_ready(out)
    dt = (time.time() - t0) / iters
    return dt * 1e9, out
